# revision 31
# baseline (speedup 1.0000x reference)
"""Multi-head attention (B=2, S=2048, EMB=1024, H=16, hd=64) on 8 TRN2 cores.

Sharding: core c -> batch b = c//4, head-group g = c%4 (4 heads, 256 emb dims).
Per core (fp16 matmuls, f32 psum):
  A) Q^T = Wq_g @ x_b^T  [256, 2048], K^T likewise (transposed layout);
     V = x_b @ Wv_g^T [2048, 256] natural layout with a ones column per head
     (row 64 of the AV accumulator then holds the softmax sums).
  B) per head-PAIR mh (heads 2mh, 2mh+1 packed on PE row halves), per
     q-window of 512, per k-tile t: S^T tile [128, 1024] (both heads) in
     psum; exp via ACT *or* DVE (int16 Schraudolph bit-trick, tunable
     fraction) -> fp16 P; AV matmuls accumulate U_aug [65, 512] per head.
  C) softmax: sums row -> SBUF-DMA bounce to partition 0 -> DVE fast recip
     -> gpsimd partition_broadcast -> DVE multiply writes O^T over qT.
  D) y partial per pair: O^T_mh @ Wo_mh -> fp16 y0/y1 outputs; pair 0's
     matmuls/evacs/DMAs are interleaved into pair 1's attention loop; host
     sums the 8 partials per batch.
"""
import os

import numpy as np

import concourse.bass as bass
import concourse.tile as tile
from concourse import bacc, mybir
from concourse.bass_utils import run_bass_kernel_spmd

F32 = mybir.dt.float32
F16 = mybir.dt.float16
I16 = mybir.dt.int16
EXP = mybir.ActivationFunctionType.Exp
MULT = mybir.AluOpType.mult
ADD = mybir.AluOpType.add

EMB = 1024
S = 2048
B = 2
HG = 4           # heads per core
HD = 64
CHD = HG * HD    # 256 emb dims per core
ET = EMB // 128  # 8 e-tiles
NT = S // 128    # 16 k-tiles
QB = 512
W = 512          # q-window for phase B
NW = S // W      # 4

# Schraudolph exp-on-DVE: i16 = f32_to_i16(s * SCH_A + SCH_B); bits as fp16.
# SCH_A folds the 1/8 temperature. SIGMA tunes the rounding/chord bias.
SCH_A = (2.0 ** 10) * 1.4426950408889634 / 8.0
SIGMA = float(os.environ.get("SCH_SIGMA", "-35.0"))
SCH_B = 15.0 * 1024.0 + SIGMA
# exp engine split: head A -> ACT; head B of DVE_N out of each 16 k-tiles
# -> DVE via Schraudolph (error scales as sqrt(DVE_N/32)), rest -> ACT.
DVE_N = int(os.environ.get("DVE_N", "16"))
# normalization multiply on gpsimd (1) or DVE (0)
GP_TT = int(os.environ.get("GP_TT", "1"))
# "headsplit": ACT does head A + DVE head B per tile; "alt": whole tiles
# alternate between engines by k-tile parity
EXP_MODE = os.environ.get("EXP_MODE", "headsplit")

_NC = None


def _dve_tile(t):
    # spread DVE head-B tiles evenly over the 16 k-tiles
    return (t * DVE_N) % NT < DVE_N


def _build():
    nc = bacc.Bacc("TRN2", target_bir_lowering=False, debug=False)
    xq_t = nc.dram_tensor("xq_t", [EMB, S], F16, kind="ExternalInput").ap()
    xk_t = nc.dram_tensor("xk_t", [EMB, S], F16, kind="ExternalInput").ap()
    xv_t = nc.dram_tensor("xv_t", [EMB, S], F16, kind="ExternalInput").ap()
    wq_t = nc.dram_tensor("wq_t", [EMB, CHD], F16, kind="ExternalInput").ap()
    wk_t = nc.dram_tensor("wk_t", [EMB, CHD], F16, kind="ExternalInput").ap()
    wv_t = nc.dram_tensor("wv_t", [EMB, CHD], F16, kind="ExternalInput").ap()
    wo_t = nc.dram_tensor("wo_t", [CHD, EMB], F16, kind="ExternalInput").ap()
    y_out = nc.dram_tensor("y", [S, EMB], F16, kind="ExternalOutput").ap()

    with tile.TileContext(nc) as tc:
        with tc.tile_pool(name="const", bufs=1) as cpool, \
             tc.tile_pool(name="big", bufs=1) as big, \
             tc.tile_pool(name="usb", bufs=2) as usb, \
             tc.tile_pool(name="pt", bufs=3) as ptp, \
             tc.tile_pool(name="yp", bufs=6) as ypool:

            wo_sb = cpool.tile([128, 2, EMB], F16, name="wo_sb")

            qT = big.tile([128, 2, S], F16, name="qT")     # later holds O^T
            kT = big.tile([128, 2, S], F16, name="kT")
            v_sb = big.tile([128, NT, HG * (HD + 1)], F16, name="v_sb")
            nc.vector.memset(v_sb[:], 1.0)                 # ones cols survive

            # ---- phase A: projections ----
            with tc.tile_pool(name="wqk", bufs=2) as wpool, \
                 tc.tile_pool(name="xp", bufs=10) as xp, \
                 tc.tile_pool(name="psA", bufs=8, space="PSUM") as psA:
                for name, xdram, wdram, dst in (
                        ("q", xq_t, wq_t, qT), ("k", xk_t, wk_t, kT)):
                    w_sb = wpool.tile([128, ET, CHD], F16, tag="w",
                                      name=f"w{name}_sb")
                    wr = wdram.rearrange("(po pi) m -> pi po m", pi=128)
                    if name == "q":
                        # per-e-tile chunks so the first matmul doesn't wait
                        # for the whole 512KB weight load
                        for e in range(ET):
                            nc.sync.dma_start(w_sb[:, e, :], wr[:, e, :])
                    else:
                        nc.sync.dma_start(w_sb[:], wr)
                    pss = [psA.tile([128, QB], F32, tag="ps", name=f"ps_{name}{i}")
                           for i in range(8)]
                    for e in range(ET):
                        x_t = xp.tile([128, S], F16, tag="x", name=f"x_{name}{e}")
                        if name == "q" and e == 0:
                            # chunked first DMA: the first matmul starts after
                            # a quarter-tile instead of the whole 512KB tile
                            for c4 in range(4):
                                nc.sync.dma_start(
                                    x_t[:, c4 * QB:(c4 + 1) * QB],
                                    xdram[0:128, c4 * QB:(c4 + 1) * QB])
                        else:
                            nc.sync.dma_start(
                                x_t[:], xdram[e * 128:(e + 1) * 128, :])
                        for m in range(2):
                            for qb in range(4):
                                nc.tensor.matmul(
                                    pss[m * 4 + qb][:],
                                    w_sb[:, e, m * 128:(m + 1) * 128],
                                    x_t[:, qb * QB:(qb + 1) * QB],
                                    start=(e == 0), stop=(e == ET - 1))
                    for m in range(2):
                        for qb in range(4):
                            cp = nc.scalar.copy if (m + qb) % 2 else \
                                nc.vector.tensor_copy
                            cp(dst[:, m, qb * QB:(qb + 1) * QB],
                               pss[m * 4 + qb][:])

                # V natural layout; all 8 xv e-tiles resident
                wv_sb = wpool.tile([128, ET, CHD], F16, tag="w", name="wv_sb")
                nc.sync.dma_start(
                    wv_sb[:], wv_t.rearrange("(po pi) m -> pi po m", pi=128))
                xv_tiles = []
                for e in range(ET):
                    x_t = xp.tile([128, S], F16, tag="x", name=f"x_v{e}")
                    nc.sync.dma_start(x_t[:], xv_t[e * 128:(e + 1) * 128, :])
                    xv_tiles.append(x_t)
                nc.sync.dma_start(
                    wo_sb[:], wo_t.rearrange("(ct p) n -> p ct n", p=128))
                for s in range(NT):
                    v_ps = psA.tile([128, CHD], F32, tag="ps", name=f"ps_v{s}")
                    for e in range(ET):
                        nc.tensor.matmul(
                            v_ps[:], xv_tiles[e][:, s * 128:(s + 1) * 128],
                            wv_sb[:, e, :],
                            start=(e == 0), stop=(e == ET - 1))
                    src = v_ps[:].rearrange("p (h d) -> p h d", d=HD)
                    dstv = v_sb[:, s, :].rearrange("p (h d) -> p h d",
                                                   d=HD + 1)[:, :, 0:HD]
                    cp = nc.scalar.copy if s % 2 else nc.vector.tensor_copy
                    cp(dstv, src)

            # ---- phases B + C + D ----
            # Softmax-normalization chains run per q-window, pipelined one
            # window behind the attention loop; pair 0's output projection is
            # interleaved into pair 1's loop. The B loop is software-pipelined
            # (scores of t+1 issue on the PE while exp of t runs on ACT/DVE).
            u_sb = {}   # (mh, head_in_pair) -> [65, S] f32 accumulators

            def emit_d_tile(s, y_ps):
                # both head pairs accumulate into one psum tile
                for nb in range(2):
                    for m in range(2):
                        nc.tensor.matmul(
                            y_ps[:, nb * QB:(nb + 1) * QB],
                            qT[:, m, s * 128:(s + 1) * 128],
                            wo_sb[:, m, nb * QB:(nb + 1) * QB],
                            start=(m == 0), stop=(m == 1))
                y_sb = ypool.tile([128, EMB], F16, tag="ysb", name=f"ysb{s}")
                cp = nc.scalar.copy if s % 2 else nc.vector.tensor_copy
                cp(y_sb[:], y_ps[:])
                # issue from the (otherwise idle) gpsimd queue: Sync's
                # descriptor generation is ~650ns each and would pace the tail
                nc.gpsimd.dma_start(y_out[s * 128:(s + 1) * 128, :], y_sb[:])

            def emit_norm_chunk(mh, qh):
                # O^T[:, qh window] = U * (1/sums) for head pair mh.
                # Broadcast r across partitions via a DRAM bounce (stride-0
                # partition reads are legal from DRAM); the chain's latency
                # hides under the next window's 16 iterations. gpsimd runs
                # ONLY tensor_tensor (mixing op types thrashes its library).
                qo = qh * W
                for h2 in range(2):
                    u = u_sb[(mh, h2)]
                    sr = rpool.tile([1, W], F32, tag=f"sr{h2}",
                                    name=f"sr{mh}_{qh}_{h2}")
                    nc.sync.dma_start(sr[:], u[HD:HD + 1, qo:qo + W])
                    rr = rpool.tile([1, W], F32, tag=f"rr{h2}",
                                    name=f"rr{mh}_{qh}_{h2}")
                    nc.vector.reciprocal_approx_fast(out=rr[:], in_=sr[:])
                    rd = rdram.tile([1, W], F32, name=f"rd{mh}_{qh}_{h2}")
                    nc.sync.dma_start(rd[:], rr[:])
                    rb = rpool.tile([HD, W], F32, tag=f"rb{h2}",
                                    name=f"rb{mh}_{qh}_{h2}")
                    nc.sync.dma_start(rb[:], rd[:].to_broadcast([HD, W]))
                    tt = nc.gpsimd.tensor_tensor if GP_TT else \
                        nc.vector.tensor_tensor
                    tt(qT[h2 * HD:(h2 + 1) * HD, mh, qo:qo + W],
                       u[0:HD, qo:qo + W], rb[:], MULT)

            with tc.tile_pool(name="rp", bufs=2) as rpool, \
                 tc.tile_pool(name="rd", bufs=4, space="DRAM") as rdram, \
                 tc.tile_pool(name="psB", bufs=3, space="PSUM") as psB, \
                 tc.tile_pool(name="psU", bufs=1, space="PSUM") as psU:
                for mh in range(2):
                    hA, hB = 2 * mh, 2 * mh + 1
                    uA = usb.tile([HD + 1, S], F32, tag="uA", name=f"uA{mh}")
                    uB = usb.tile([HD + 1, S], F32, tag="uB", name=f"uB{mh}")
                    u_sb[(mh, 0)], u_sb[(mh, 1)] = uA, uB

                    def emit_scores(mh, qh, t):
                        # separate per-head tiles: no shared-tile ordering
                        # coupling between the two heads' exp/AV chains
                        sps = []
                        qo = qh * W
                        for hx, bp in ((0, 0), (1, 64)):
                            sp = psB.tile([128, W], F32, tag=f"sp{hx}",
                                          name=f"sp{hx}_{mh}{qh}{t}")
                            nc.tensor.matmul(
                                sp[:],
                                kT[bp:bp + HD, mh, t * 128:(t + 1) * 128],
                                qT[bp:bp + HD, mh, qo:qo + W],
                                start=True, stop=True)
                            sps.append(sp)
                        return sps

                    sp_cur = None
                    for qh in range(NW):
                        uaccA = psU.tile([HD + 1, W], F32, tag="uaccA",
                                         name=f"uaccA{mh}_{qh}")
                        uaccB = psU.tile([HD + 1, W], F32, tag="uaccB",
                                         name=f"uaccB{mh}_{qh}")
                        if sp_cur is None:
                            sp_cur = emit_scores(mh, qh, 0)
                        if qh > 0:
                            # previous window's softmax chain (latency hides
                            # under this window's 16 iterations)
                            emit_norm_chunk(mh, qh - 1)
                        for t in range(NT):
                            # next iteration's scores keep the PE busy while
                            # this tile's exp runs
                            sp = sp_cur
                            if t + 1 < NT:
                                sp_cur = emit_scores(mh, qh, t + 1)
                            elif qh + 1 < NW:
                                sp_cur = emit_scores(mh, qh + 1, 0)
                            else:
                                sp_cur = None
                            spA, spB = sp
                            pCA = ptp.tile([128, W], F16, tag="pcA",
                                           name=f"pcA{mh}{qh}{t}")
                            pCB = ptp.tile([128, W], F16, tag="pcB",
                                           name=f"pcB{mh}{qh}{t}")
                            # per-head exp: AV of a head only waits its own
                            # 720ns half, and ACT/DVE overlap
                            nc.scalar.activation(pCA[:], spA[:], EXP,
                                                 scale=0.125)
                            if _dve_tile(t):
                                nc.vector.tensor_scalar(
                                    pCB[:].bitcast(I16), spB[:],
                                    SCH_A, SCH_B, MULT, ADD)
                            else:
                                nc.scalar.activation(pCB[:], spB[:], EXP,
                                                     scale=0.125)
                            for h2, uacc, p_t in ((hA, uaccA, pCA),
                                                  (hB, uaccB, pCB)):
                                nc.tensor.matmul(
                                    uacc[:],
                                    v_sb[:, t, h2 * (HD + 1):
                                         (h2 + 1) * (HD + 1)],
                                    p_t[:],
                                    start=(t == 0), stop=(t == NT - 1))
                        nc.scalar.copy(uA[:, qh * W:(qh + 1) * W], uaccA[:])
                        nc.vector.tensor_copy(uB[:, qh * W:(qh + 1) * W],
                                              uaccB[:])
                    emit_norm_chunk(mh, NW - 1)

            # ---- phase D tail: both pairs' output projections ----
            with tc.tile_pool(name="psY1", bufs=4, space="PSUM") as psY1:
                # keep the PE warm across the pair-1 tail normalization chain
                trash = psY1.tile([128, EMB], F32, tag="yps", name="warm")
                for wi in range(14):
                    nc.tensor.matmul(trash[:, 0:QB], v_sb[:, 0, 0:128],
                                     v_sb[:, 0:2, 0:256], start=True, stop=True)
                for s in range(NT):
                    y_ps = psY1.tile([128, EMB], F32, tag="yps", name=f"yps{s}")
                    emit_d_tile(s, y_ps)

    nc.compile()
    return nc


def get_nc():
    global _NC
    if _NC is None:
        _NC = _build()
    return _NC


def make_in_maps(query, key, value, Wq, Wk, Wv, Wo):
    query = np.asarray(query, dtype=np.float32)
    key = np.asarray(key, dtype=np.float32)
    value = np.asarray(value, dtype=np.float32)
    Wq = np.asarray(Wq, dtype=np.float32)
    Wk = np.asarray(Wk, dtype=np.float32)
    Wv = np.asarray(Wv, dtype=np.float32)
    Wo = np.asarray(Wo, dtype=np.float32)
    xt = {(n, b): np.ascontiguousarray(x[b].T).astype(np.float16)
          for n, x in (("q", query), ("k", key), ("v", value))
          for b in range(B)}
    in_maps = []
    for c in range(8):
        b, g = divmod(c, 4)
        hs = slice(g * CHD, (g + 1) * CHD)
        in_maps.append({
            "xq_t": xt[("q", b)],
            "xk_t": xt[("k", b)],
            "xv_t": xt[("v", b)],
            "wq_t": np.ascontiguousarray(Wq[hs, :].T).astype(np.float16),
            "wk_t": np.ascontiguousarray(Wk[hs, :].T).astype(np.float16),
            "wv_t": np.ascontiguousarray(Wv[hs, :].T).astype(np.float16),
            "wo_t": np.ascontiguousarray(Wo[:, hs].T).astype(np.float16),
        })
    return in_maps


def gather(results):
    out = np.zeros((B, S, EMB), dtype=np.float32)
    for c in range(8):
        out[c // 4] += results[c]["y"].astype(np.float32)
    return out


def kernel(**inputs) -> np.ndarray:
    nc = get_nc()
    in_maps = make_in_maps(**inputs)
    res = run_bass_kernel_spmd(nc, in_maps, core_ids=list(range(8)))
    return gather(res.results)


# revision 33
# speedup vs baseline: 1.0661x; 1.0661x over previous
"""Multi-head attention (B=2, S=2048, EMB=1024, H=16, hd=64) on 8 TRN2 cores.

Sharding: core c -> batch b = c//4, head-group g = c%4 (4 heads, 256 emb dims).
Per core (fp16 matmuls, f32 psum):
  A) Q^T = Wq_g @ x_b^T  [256, 2048], K^T likewise (transposed layout);
     V = x_b @ Wv_g^T [2048, 256] natural layout with a ones column per head
     (row 64 of the AV accumulator then holds the softmax sums).
  B) per head-PAIR mh (heads 2mh, 2mh+1 packed on PE row halves), per
     q-window of 512, per k-tile t: S^T tile [128, 1024] (both heads) in
     psum; exp via ACT *or* DVE (int16 Schraudolph bit-trick, tunable
     fraction) -> fp16 P; AV matmuls accumulate U_aug [65, 512] per head.
  C) softmax: sums row -> SBUF-DMA bounce to partition 0 -> DVE fast recip
     -> gpsimd partition_broadcast -> DVE multiply writes O^T over qT.
  D) y partial per pair: O^T_mh @ Wo_mh -> fp16 y0/y1 outputs; pair 0's
     matmuls/evacs/DMAs are interleaved into pair 1's attention loop; host
     sums the 8 partials per batch.
"""
import os

import numpy as np

import concourse.bass as bass
import concourse.tile as tile
from concourse import bacc, mybir
from concourse.bass_utils import run_bass_kernel_spmd

F32 = mybir.dt.float32
F16 = mybir.dt.float16
I16 = mybir.dt.int16
EXP = mybir.ActivationFunctionType.Exp
MULT = mybir.AluOpType.mult
ADD = mybir.AluOpType.add

EMB = 1024
S = 2048
B = 2
HG = 4           # heads per core
HD = 64
CHD = HG * HD    # 256 emb dims per core
ET = EMB // 128  # 8 e-tiles
NT = S // 128    # 16 k-tiles
QB = 512
W = 512          # q-window for phase B
NW = S // W      # 4

# Schraudolph exp-on-DVE: i16 = f32_to_i16(s * SCH_A + SCH_B); bits as fp16.
# SCH_A folds the 1/8 temperature. SIGMA tunes the rounding/chord bias.
SCH_A = (2.0 ** 10) * 1.4426950408889634 / 8.0
SIGMA = float(os.environ.get("SCH_SIGMA", "-35.0"))
SCH_B = 15.0 * 1024.0 + SIGMA
# exp engine split: head A -> ACT; head B of DVE_N out of each 16 k-tiles
# -> DVE via Schraudolph (error scales as sqrt(DVE_N/32)), rest -> ACT.
DVE_N = int(os.environ.get("DVE_N", "16"))
# normalization multiply on gpsimd (1) or DVE (0)
GP_TT = int(os.environ.get("GP_TT", "1"))
# "headsplit": ACT does head A + DVE head B per tile; "alt": whole tiles
# alternate between engines by k-tile parity
EXP_MODE = os.environ.get("EXP_MODE", "headsplit")

_NC = None


def _dve_tile(t):
    # spread DVE head-B tiles evenly over the 16 k-tiles
    return (t * DVE_N) % NT < DVE_N


def _build():
    nc = bacc.Bacc("TRN2", target_bir_lowering=False, debug=False)
    xq_t = nc.dram_tensor("xq_t", [EMB, S], F16, kind="ExternalInput").ap()
    xk_t = nc.dram_tensor("xk_t", [EMB, S], F16, kind="ExternalInput").ap()
    xv_t = nc.dram_tensor("xv_t", [EMB, S], F16, kind="ExternalInput").ap()
    wq_t = nc.dram_tensor("wq_t", [EMB, CHD], F16, kind="ExternalInput").ap()
    wk_t = nc.dram_tensor("wk_t", [EMB, CHD], F16, kind="ExternalInput").ap()
    wv_t = nc.dram_tensor("wv_t", [EMB, CHD], F16, kind="ExternalInput").ap()
    wo_t = nc.dram_tensor("wo_t", [CHD, EMB], F16, kind="ExternalInput").ap()
    y_out = nc.dram_tensor("y", [S, EMB], F16, kind="ExternalOutput").ap()

    with tile.TileContext(nc) as tc:
        with tc.tile_pool(name="const", bufs=1) as cpool, \
             tc.tile_pool(name="big", bufs=1) as big, \
             tc.tile_pool(name="usb", bufs=2) as usb, \
             tc.tile_pool(name="pt", bufs=3) as ptp, \
             tc.tile_pool(name="yp", bufs=6) as ypool:

            wo_sb = cpool.tile([128, 2, EMB], F16, name="wo_sb")

            qT = big.tile([128, 2, S], F16, name="qT")     # later holds O^T
            kT = big.tile([128, 2, S], F16, name="kT")
            v_sb = big.tile([128, NT, HG * (HD + 1)], F16, name="v_sb")
            nc.vector.memset(v_sb[:], 1.0)                 # ones cols survive

            # ---- phase A: projections ----
            with tc.tile_pool(name="wqk", bufs=2) as wpool, \
                 tc.tile_pool(name="xp", bufs=10) as xp, \
                 tc.tile_pool(name="psA", bufs=8, space="PSUM") as psA:
                for name, xdram, wdram, dst in (
                        ("q", xq_t, wq_t, qT), ("k", xk_t, wk_t, kT)):
                    w_sb = wpool.tile([128, ET, CHD], F16, tag="w",
                                      name=f"w{name}_sb")
                    wr = wdram.rearrange("(po pi) m -> pi po m", pi=128)
                    if name == "q":
                        # per-e-tile chunks so the first matmul doesn't wait
                        # for the whole 512KB weight load
                        for e in range(ET):
                            nc.sync.dma_start(w_sb[:, e, :], wr[:, e, :])
                    else:
                        nc.sync.dma_start(w_sb[:], wr)
                    pss = [psA.tile([128, QB], F32, tag="ps", name=f"ps_{name}{i}")
                           for i in range(8)]
                    for e in range(ET):
                        x_t = xp.tile([128, S], F16, tag="x", name=f"x_{name}{e}")
                        if name == "q" and e == 0:
                            # chunked first DMA: the first matmul starts after
                            # a quarter-tile instead of the whole 512KB tile
                            for c4 in range(4):
                                nc.sync.dma_start(
                                    x_t[:, c4 * QB:(c4 + 1) * QB],
                                    xdram[0:128, c4 * QB:(c4 + 1) * QB])
                        else:
                            nc.sync.dma_start(
                                x_t[:], xdram[e * 128:(e + 1) * 128, :])
                        for m in range(2):
                            for qb in range(4):
                                nc.tensor.matmul(
                                    pss[m * 4 + qb][:],
                                    w_sb[:, e, m * 128:(m + 1) * 128],
                                    x_t[:, qb * QB:(qb + 1) * QB],
                                    start=(e == 0), stop=(e == ET - 1))
                    for m in range(2):
                        for qb in range(4):
                            cp = nc.scalar.copy if (m + qb) % 2 else \
                                nc.vector.tensor_copy
                            cp(dst[:, m, qb * QB:(qb + 1) * QB],
                               pss[m * 4 + qb][:])

                # V natural layout; all 8 xv e-tiles resident
                wv_sb = wpool.tile([128, ET, CHD], F16, tag="w", name="wv_sb")
                nc.sync.dma_start(
                    wv_sb[:], wv_t.rearrange("(po pi) m -> pi po m", pi=128))
                xv_tiles = []
                for e in range(ET):
                    x_t = xp.tile([128, S], F16, tag="x", name=f"x_v{e}")
                    nc.sync.dma_start(x_t[:], xv_t[e * 128:(e + 1) * 128, :])
                    xv_tiles.append(x_t)
                nc.sync.dma_start(
                    wo_sb[:], wo_t.rearrange("(ct p) n -> p ct n", p=128))
                for s in range(NT):
                    v_ps = psA.tile([128, CHD], F32, tag="ps", name=f"ps_v{s}")
                    for e in range(ET):
                        nc.tensor.matmul(
                            v_ps[:], xv_tiles[e][:, s * 128:(s + 1) * 128],
                            wv_sb[:, e, :],
                            start=(e == 0), stop=(e == ET - 1))
                    src = v_ps[:].rearrange("p (h d) -> p h d", d=HD)
                    dstv = v_sb[:, s, :].rearrange("p (h d) -> p h d",
                                                   d=HD + 1)[:, :, 0:HD]
                    cp = nc.scalar.copy if s % 2 else nc.vector.tensor_copy
                    cp(dstv, src)

            # ---- phases B + C + D ----
            # Softmax-normalization chains run per q-window, pipelined one
            # window behind the attention loop; pair 0's output projection is
            # interleaved into pair 1's loop. The B loop is software-pipelined
            # (scores of t+1 issue on the PE while exp of t runs on ACT/DVE).
            u_sb = {}   # (mh, head_in_pair) -> [65, S] f32 accumulators

            def emit_d_tile(s, y_ps):
                # both head pairs accumulate into one psum tile
                for nb in range(2):
                    for m in range(2):
                        nc.tensor.matmul(
                            y_ps[:, nb * QB:(nb + 1) * QB],
                            qT[:, m, s * 128:(s + 1) * 128],
                            wo_sb[:, m, nb * QB:(nb + 1) * QB],
                            start=(m == 0), stop=(m == 1))
                y_sb = ypool.tile([128, EMB], F16, tag="ysb", name=f"ysb{s}")
                cp = nc.scalar.copy if s % 2 else nc.vector.tensor_copy
                cp(y_sb[:], y_ps[:])
                # issue from the (otherwise idle) gpsimd queue: Sync's
                # descriptor generation is ~650ns each and would pace the tail
                nc.gpsimd.dma_start(y_out[s * 128:(s + 1) * 128, :], y_sb[:])

            def emit_norm_chunk(mh, qh):
                # O^T[:, qh window] = U * (1/sums) for head pair mh.
                # Broadcast r across partitions via a DRAM bounce (stride-0
                # partition reads are legal from DRAM); the chain's latency
                # hides under the next window's 16 iterations. gpsimd runs
                # ONLY tensor_tensor (mixing op types thrashes its library).
                qo = qh * W
                for h2 in range(2):
                    u = u_sb[(mh, h2)]
                    sr = rpool.tile([1, W], F32, tag=f"sr{h2}",
                                    name=f"sr{mh}_{qh}_{h2}")
                    nc.sync.dma_start(sr[:], u[HD:HD + 1, qo:qo + W])
                    rr = rpool.tile([1, W], F32, tag=f"rr{h2}",
                                    name=f"rr{mh}_{qh}_{h2}")
                    nc.vector.reciprocal_approx_fast(out=rr[:], in_=sr[:])
                    rd = rdram.tile([1, W], F32, name=f"rd{mh}_{qh}_{h2}")
                    nc.sync.dma_start(rd[:], rr[:])
                    rb = rpool.tile([HD, W], F32, tag=f"rb{h2}",
                                    name=f"rb{mh}_{qh}_{h2}")
                    nc.sync.dma_start(rb[:], rd[:].to_broadcast([HD, W]))
                    tt = nc.gpsimd.tensor_tensor if GP_TT else \
                        nc.vector.tensor_tensor
                    tt(qT[h2 * HD:(h2 + 1) * HD, mh, qo:qo + W],
                       u[0:HD, qo:qo + W], rb[:], MULT)

            with tc.tile_pool(name="rp", bufs=2) as rpool, \
                 tc.tile_pool(name="rd", bufs=4, space="DRAM") as rdram, \
                 tc.tile_pool(name="psB", bufs=3, space="PSUM") as psB, \
                 tc.tile_pool(name="psU", bufs=1, space="PSUM") as psU:
                for mh in range(2):
                    hA, hB = 2 * mh, 2 * mh + 1
                    uA = usb.tile([HD + 1, S], F32, tag="uA", name=f"uA{mh}")
                    uB = usb.tile([HD + 1, S], F32, tag="uB", name=f"uB{mh}")
                    u_sb[(mh, 0)], u_sb[(mh, 1)] = uA, uB

                    def emit_scores(mh, qh, t):
                        sp = psB.tile([128, 2 * W], F32, tag="sp",
                                      name=f"sp{mh}{qh}{t}")
                        qo = qh * W
                        for bp, co in ((0, 0), (64, W)):
                            nc.tensor.matmul(
                                sp[:, co:co + W],
                                kT[bp:bp + HD, mh, t * 128:(t + 1) * 128],
                                qT[bp:bp + HD, mh, qo:qo + W],
                                start=True, stop=True)
                        return sp

                    sp_cur = None
                    for qh in range(NW):
                        uaccA = psU.tile([HD + 1, W], F32, tag="uaccA",
                                         name=f"uaccA{mh}_{qh}")
                        uaccB = psU.tile([HD + 1, W], F32, tag="uaccB",
                                         name=f"uaccB{mh}_{qh}")
                        if sp_cur is None:
                            sp_cur = emit_scores(mh, qh, 0)
                        if qh > 0:
                            # previous window's softmax chain (latency hides
                            # under this window's 16 iterations)
                            emit_norm_chunk(mh, qh - 1)
                        for t in range(NT):
                            # next iteration's scores keep the PE busy while
                            # this tile's exp runs
                            sp = sp_cur
                            if t + 1 < NT:
                                sp_cur = emit_scores(mh, qh, t + 1)
                            elif qh + 1 < NW:
                                sp_cur = emit_scores(mh, qh + 1, 0)
                            else:
                                sp_cur = None
                            pC = ptp.tile([128, 2 * W], F16, tag="pc",
                                          name=f"pc{mh}{qh}{t}")
                            # per-head exp: AV of a head only waits its own
                            # 720ns half, and ACT/DVE overlap
                            nc.scalar.activation(pC[:, 0:W], sp[:, 0:W], EXP,
                                                 scale=0.125)
                            if _dve_tile(t):
                                nc.vector.tensor_scalar(
                                    pC[:, W:2 * W].bitcast(I16),
                                    sp[:, W:2 * W],
                                    SCH_A, SCH_B, MULT, ADD)
                            else:
                                nc.scalar.activation(pC[:, W:2 * W],
                                                     sp[:, W:2 * W], EXP,
                                                     scale=0.125)
                            for h2, uacc, co in ((hA, uaccA, 0),
                                                 (hB, uaccB, W)):
                                nc.tensor.matmul(
                                    uacc[:],
                                    v_sb[:, t, h2 * (HD + 1):
                                         (h2 + 1) * (HD + 1)],
                                    pC[:, co:co + W],
                                    start=(t == 0), stop=(t == NT - 1))
                        nc.scalar.copy(uA[:, qh * W:(qh + 1) * W], uaccA[:])
                        nc.vector.tensor_copy(uB[:, qh * W:(qh + 1) * W],
                                              uaccB[:])
                    emit_norm_chunk(mh, NW - 1)

            # ---- phase D tail: both pairs' output projections ----
            with tc.tile_pool(name="psY1", bufs=4, space="PSUM") as psY1:
                # keep the PE warm across the pair-1 tail normalization chain
                trash = psY1.tile([128, EMB], F32, tag="yps", name="warm")
                for wi in range(14):
                    nc.tensor.matmul(trash[:, 0:QB], v_sb[:, 0, 0:128],
                                     v_sb[:, 0:2, 0:256], start=True, stop=True)
                for s in range(NT):
                    y_ps = psY1.tile([128, EMB], F32, tag="yps", name=f"yps{s}")
                    emit_d_tile(s, y_ps)

    nc.compile()
    return nc


def get_nc():
    global _NC
    if _NC is None:
        _NC = _build()
    return _NC


def make_in_maps(query, key, value, Wq, Wk, Wv, Wo):
    query = np.asarray(query, dtype=np.float32)
    key = np.asarray(key, dtype=np.float32)
    value = np.asarray(value, dtype=np.float32)
    Wq = np.asarray(Wq, dtype=np.float32)
    Wk = np.asarray(Wk, dtype=np.float32)
    Wv = np.asarray(Wv, dtype=np.float32)
    Wo = np.asarray(Wo, dtype=np.float32)
    xt = {(n, b): np.ascontiguousarray(x[b].T).astype(np.float16)
          for n, x in (("q", query), ("k", key), ("v", value))
          for b in range(B)}
    in_maps = []
    for c in range(8):
        b, g = divmod(c, 4)
        hs = slice(g * CHD, (g + 1) * CHD)
        in_maps.append({
            "xq_t": xt[("q", b)],
            "xk_t": xt[("k", b)],
            "xv_t": xt[("v", b)],
            "wq_t": np.ascontiguousarray(Wq[hs, :].T).astype(np.float16),
            "wk_t": np.ascontiguousarray(Wk[hs, :].T).astype(np.float16),
            "wv_t": np.ascontiguousarray(Wv[hs, :].T).astype(np.float16),
            "wo_t": np.ascontiguousarray(Wo[:, hs].T).astype(np.float16),
        })
    return in_maps


def gather(results):
    out = np.zeros((B, S, EMB), dtype=np.float32)
    for c in range(8):
        out[c // 4] += results[c]["y"].astype(np.float32)
    return out


def kernel(**inputs) -> np.ndarray:
    nc = get_nc()
    in_maps = make_in_maps(**inputs)
    res = run_bass_kernel_spmd(nc, in_maps, core_ids=list(range(8)))
    return gather(res.results)
